# revision 39
# baseline (speedup 1.0000x reference)
"""Trainium2 Bass kernel for a top-2 MoE classifier head (B=4096, D=1024,
E=16 experts, H=2048, C=1000 classes, k=2), expert-parallel over 8 NeuronCores.

v6 strategy (per core i of 8):
  - host passes xT (fp32 [D,B]) so the fp32 gate matmul needs NO on-device
    transposes, and x in bf16 [B,D] for the expert gather.
  - softmax + top-2 + local-expert masks as batched [128,NT,E] DVE ops;
    compaction is MATMUL-based: rank = prefix-scan of the membership mask,
    split into hi/lo 7-bit parts; ONE [128,NT,128] one-hot per expert (lo),
    hi folded into a tiny bf16 metadata rhs; 32 N=20 matmuls per expert pick
    each slot's (f, p, wv, occupied) straight into PSUM.  No indirect
    scatters, no DRAM round trip.
  - expert FFN: indirect-gather bf16 x rows, one XBAR DMA-transpose per
    128-slot tile (out[p,c,s] = in[s, c*128+p]) into a contiguous staging
    tile, DVE-copy into [P, DC, CAP] xdt; fc1/fc2 in bf16 with fp32 PSUM,
    weights double-buffered in halves; fc1 PSUM drains (+bias) on ACT;
    log_softmax fp32; scatter-add weighted rows (bf16) into a DRAM partial.
  - ReduceScatter(sum, bf16) leaves each core the final [512,1000] slice;
    host concatenates and upcasts.
  - bodies are SOFTWARE-PIPELINED: body k+1's gate matmuls are emitted
    inside body k's fc1(e0), its routing DVE inside fc1(e1), and its
    compaction/gathers before fc2 — so in steady state the PE never waits
    for routing, and the collective hides under the next body's FFN.
"""

import numpy as np

import bass_rust
import concourse.bass as bass
import concourse.mybir as mybir
import concourse.tile as tile
from concourse.bass_utils import run_bass_kernel_spmd
from concourse.vector_clock import ScopedClock

# ---------------------------------------------------------------- constants
B, D, E, H, C, TOPK = 4096, 1024, 16, 2048, 1000, 2
N_CORES = 8
P = 128
NT = B // P            # 32 token tiles
DC = D // P            # 8 d-chunks
HC = H // P            # 16 h-chunks
ELOC = E // N_CORES    # 2 experts per core
CAP = 640              # capacity (slots) per expert; true load ~512 +- 25
CAPT = CAP // P        # 5 slot tiles per expert
TRASH = B              # scatter target for padding rows (row B of partial)
NW = 16                # gate windows
W = B // NW            # 256 tokens per window
FP32 = mybir.dt.float32
BF16 = mybir.dt.bfloat16
FP16 = mybir.dt.float16
INT32 = mybir.dt.int32

_COMBINE = "rs"        # "rs" = on-device ReduceScatter; "hostsum" = host combine


# ------------------------------------------------- walrus compatibility patch
# This neuronxcc/walrus build rejects sync waits on CTRL-class instructions
# (Drain/NoOp) beyond a tiny cap.  Hoist every wait onto its own single-wait
# NoOp immediately before the owning instruction (same engine, program order).
def _install_walrus_compat():
    if getattr(bass.Bass, "_moe_walrus_patched", False):
        return
    bass.Bass._orig_meb = bass.Bass.multi_engine_barrier
    tile.TileContext._orig_dab = tile.TileContext._drain_and_barrier

    def _patched_meb(self, engines):
        for inst in self._multi_engine_barrier_insts(engines):
            eng = self.engines[inst.engine]
            si = inst.sync_info
            if si is not None and si.on_wait:
                for w in list(si.on_wait):
                    nop = eng.nop(nofuse=True)
                    nop.ins.sync_info = bass_rust.SyncInfo(on_wait=[w], on_update=[])
                inst.sync_info = bass_rust.SyncInfo(
                    on_wait=[], on_update=list(si.on_update)
                )
            eng.add_instruction(inst)

    bass.Bass.multi_engine_barrier = _patched_meb
    _patched_meb_ref[0] = _patched_meb

    def _patched_dab(self, tick_clock, wait_clock):
        drain_inst = self.nc.sync.drain()
        wait_clock.add_sem_waits(
            drain_inst.ins, ScopedClock({None: tick_clock.global_clock})
        )
        si = drain_inst.ins.sync_info
        if si is not None and si.on_wait:
            waits = list(si.on_wait)
            drain_inst.ins.sync_info = bass_rust.SyncInfo(
                on_wait=[], on_update=list(si.on_update)
            )
            for w in waits:
                nop = self.nc.sync.nop(nofuse=True)
                nop.ins.sync_info = bass_rust.SyncInfo(on_wait=[w], on_update=[])
            self.nc.sync.drain()
        self.nc.all_engine_barrier()
        popped = self.nc._tile_sem_poison_stack.pop()
        assert popped is self._sem_poison
        self.nc.clear_and_free_semaphores(list(self.sems.allocated().values()))
        self.nc.all_engine_barrier()

    tile.TileContext._drain_and_barrier = _patched_dab
    _patched_dab_ref[0] = _patched_dab
    bass.Bass._moe_walrus_patched = True


_patched_meb_ref = [None]
_patched_dab_ref = [None]
_waitnop_ctr = [0]


def _split_excess_waits(nc):
    """Walrus compat, part 2: cap each instruction at one sync wait (none on
    Drain); excess waits move to single-wait NoOps inserted before, same
    engine."""
    for bb in nc.main_func.blocks:
        new = []
        changed = False
        for inst in bb.instructions:
            si = inst.sync_info
            if si is not None and si.on_wait:
                waits = list(si.on_wait)
                keep_budget = 0 if type(inst).__name__ == "InstDrain" else 1
                if len(waits) > keep_budget:
                    keep = waits[:keep_budget]
                    for w in waits[keep_budget:]:
                        _waitnop_ctr[0] += 1
                        nop = mybir.InstNoOp(
                            name=f"I-waitnop-{_waitnop_ctr[0]}", ins=[], outs=[]
                        )
                        nop.engine = inst.engine
                        nop.sync_info = bass_rust.SyncInfo(
                            on_wait=[w], on_update=[]
                        )
                        new.append(nop)
                    inst.sync_info = bass_rust.SyncInfo(
                        on_wait=keep, on_update=list(si.on_update)
                    )
                    changed = True
            new.append(inst)
        if changed:
            bb.instructions = new


# ------------------------------------------------------------- kernel builder
def build_bass(combine: str = _COMBINE, repeat: int = 1, for_sim: bool = False,
               upto: str = "full") -> bass.Bass:
    _install_walrus_compat()
    if for_sim:
        bass.Bass.multi_engine_barrier = bass.Bass._orig_meb
        tile.TileContext._drain_and_barrier = tile.TileContext._orig_dab
    try:
        return _build_bass_inner(combine, repeat, for_sim, upto)
    finally:
        if for_sim:
            bass.Bass.multi_engine_barrier = _patched_meb_ref[0]
            tile.TileContext._drain_and_barrier = _patched_dab_ref[0]


class _Env:
    pass


def _build_bass_inner(combine, repeat, for_sim, upto) -> bass.Bass:
    from contextlib import ExitStack

    nc = bass.Bass()
    env = _Env()
    env.nc = nc
    env.combine = combine
    env.upto = upto
    env.PDT = BF16 if combine == "rs" else FP32
    env.TT = mybir.AluOpType

    env.xt_in = nc.dram_tensor("xt", [D, B], FP32, kind="ExternalInput")
    env.xb_in = nc.dram_tensor("xb", [B, D], BF16, kind="ExternalInput")
    env.wg_in = nc.dram_tensor("wg", [D, E], FP32, kind="ExternalInput")
    env.bg_in = nc.dram_tensor("bg", [P, E], FP32, kind="ExternalInput")
    env.w1_in = nc.dram_tensor("w1", [ELOC, D, H], BF16, kind="ExternalInput")
    env.b1_in = nc.dram_tensor("b1", [ELOC, H], FP32, kind="ExternalInput")
    env.w2_in = nc.dram_tensor("w2", [ELOC, H, C], BF16, kind="ExternalInput")
    env.b2_in = nc.dram_tensor("b2", [ELOC, P, C], FP32, kind="ExternalInput")
    env.sel_in = nc.dram_tensor("sel", [ELOC, P, E], FP32, kind="ExternalInput")
    if combine == "rs":
        out_t = nc.dram_tensor("out", [B // N_CORES, C], BF16,
                               kind="ExternalOutput")
    else:
        out_t = nc.dram_tensor("out", [B, C], FP32, kind="ExternalOutput")
    env.out_t = out_t

    with tile.TileContext(nc) as tc:
        env.tc = tc
        if repeat == 0:
            with tc.tile_pool(name="z", bufs=1) as zp:
                zt = zp.tile([P, C], out_t.dtype)
                nc.vector.memset(zt[:], 0.0)
                for i in range(out_t.shape[0] // P):
                    nc.sync.dma_start(out_t[i * P:(i + 1) * P, :], zt[:])
        else:
            with ExitStack() as hctx:
                ec = hctx.enter_context
                env.dram = ec(tc.tile_pool(name="dram", bufs=2, space="DRAM"))
                env.consts = ec(tc.tile_pool(name="consts", bufs=1))
                env.persist = ec(tc.tile_pool(name="persist", bufs=2))
                env.w1p = ec(tc.tile_pool(name="w1p", bufs=2))
                env.w2p = ec(tc.tile_pool(name="w2p", bufs=2))
                env.xdtp = ec(tc.tile_pool(name="xdtp", bufs=2))
                env.xwp = ec(tc.tile_pool(name="xwp", bufs=2))
                env.gatep = ec(tc.tile_pool(name="gatep", bufs=1))
                env.routep = ec(tc.tile_pool(name="routep", bufs=1))
                env.htp = ec(tc.tile_pool(name="htp", bufs=2))
                env.ffnp = ec(tc.tile_pool(name="ffnp", bufs=2))
                env.psum = ec(tc.tile_pool(name="psum", bufs=2, space="PSUM"))
                env.pse = ec(tc.tile_pool(name="pse", bufs=1, space="PSUM"))
                env.psg = ec(tc.tile_pool(name="psg", bufs=1, space="PSUM"))

                _emit_consts(env)
                fprev = None
                rsout = None
                for _ in range(repeat):
                    if fprev is None:
                        g = _emit_gate(env)
                        fprev = _emit_route(env, g)
                        if fprev is None:        # upto == "A"
                            continue
                        continue
                    _emit_fc1(env, fprev, 0)
                    g = _emit_gate(env)
                    _emit_fc1(env, fprev, 1)
                    fnext = _emit_route(env, g)
                    rsout = _emit_fc2_combine(env, fprev)
                    fprev = fnext
                if fprev is not None and upto not in ("A", "B"):
                    _emit_fc1(env, fprev, 0)
                    _emit_fc1(env, fprev, 1)
                    rsout = _emit_fc2_combine(env, fprev)
                if rsout is not None:
                    nc.sync.dma_start(out_t[:], rsout[:])
    if not for_sim:
        _split_excess_waits(nc)
    return nc


def _emit_consts(env):
    nc, consts, TT = env.nc, env.consts, env.TT
    env.wgsb = consts.tile([P, DC, E], FP32, tag="wgsb", name="wgsb")
    nc.sync.dma_start(env.wgsb[:], env.wg_in[:].rearrange("(c p) e -> p c e", p=P))
    env.bgbc = consts.tile([P, E], FP32, tag="bgbc", name="bgbc")
    nc.sync.dma_start(env.bgbc[:], env.bg_in[:])

    # strict upper-triangular ones for the cross-partition exclusive prefix
    tri_i = consts.tile([P, P], INT32, tag="tri_i", name="tri_i")
    nc.gpsimd.iota(tri_i[:], pattern=[[1, P]], base=0, channel_multiplier=-1)
    env.tri = consts.tile([P, P], FP32, tag="tri", name="tri")
    nc.vector.tensor_scalar(env.tri[:], tri_i[:], 0, None, op0=TT.is_gt)

    env.selbc = consts.tile([P, ELOC, E], FP32, tag="selbc", name="selbc")
    nc.scalar.dma_start(env.selbc[:], env.sel_in[:].rearrange("j p e -> p j e"))
    env.b1sb = consts.tile([P, ELOC, HC], FP32, tag="b1sb", name="b1sb")
    nc.scalar.dma_start(
        env.b1sb[:], env.b1_in[:].rearrange("j (m p) -> p j m", p=P))
    env.b2bc = consts.tile([P, ELOC, C], FP32, tag="b2bc", name="b2bc")
    nc.scalar.dma_start(env.b2bc[:], env.b2_in[:].rearrange("j p c -> p j c"))

    # iotas: f index, p index, slot-within-tile
    env.fio_i = consts.tile([P, NT], INT32, tag="fio_i", name="fio_i")
    nc.gpsimd.iota(env.fio_i[:], pattern=[[1, NT]], base=0, channel_multiplier=0)
    env.pio_i = consts.tile([P, NT], INT32, tag="pio_i", name="pio_i")
    nc.gpsimd.iota(env.pio_i[:], pattern=[[0, NT]], base=0, channel_multiplier=1)
    sio_i = consts.tile([P, P], INT32, tag="sio_i", name="sio_i")
    nc.gpsimd.iota(sio_i[:], pattern=[[1, P]], base=0, channel_multiplier=0)
    env.sio16 = consts.tile([P, P], FP16, tag="sio16", name="sio16")
    nc.vector.tensor_copy(env.sio16[:], sio_i[:])

    nzr = 2 if env.combine == "rs" else 1
    env.zsb = consts.tile([P, nzr, C], env.PDT, tag="zsb", name="zsb")
    nc.vector.memset(env.zsb[:], 0.0)
    env.nzr = nzr


def _emit_gate(env):
    """Gate logits for all B tokens -> glogA [P, NT, E] (fp32)."""
    nc = env.nc
    glogA = env.gatep.tile([P, NT, E], FP32, tag="glogA", name="glogA")
    WT = W // P
    xt_v = env.xt_in[:].rearrange("(c p) t -> p c t", p=P)
    for w in range(NW):
        xw = env.xwp.tile([P, DC, W], FP32, tag="xw", name="xw")
        # split each window across BOTH HWDGE rings (d-chunk halves)
        nc.sync.dma_start(
            xw[:, :DC // 2, :], xt_v[:, :DC // 2, w * W:(w + 1) * W])
        nc.scalar.dma_start(
            xw[:, DC // 2:, :], xt_v[:, DC // 2:, w * W:(w + 1) * W])
        gps = env.psg.tile([P, WT, E], FP32, tag="gps", name="gps")
        for f2 in range(WT):
            for c in range(DC):
                nc.tensor.matmul(
                    gps[:, f2, :], lhsT=xw[:, c, f2 * P:(f2 + 1) * P],
                    rhs=env.wgsb[:, c, :],
                    start=(c == 0), stop=(c == DC - 1),
                )
        nc.scalar.copy(glogA[:, w * WT:(w + 1) * WT, :], gps[:])
    return glogA


def _emit_route(env, glogA):
    """softmax/top-2/masks, matmul compaction, gathers+transposes into xdt,
    this body's weight loads and partial zero-init.  Returns the fstate."""
    nc, TT = env.nc, env.TT
    route_p, gate_p = env.routep, env.gatep

    def bc_last(ap2d):
        return ap2d.unsqueeze(2).broadcast_to([P, NT, E])

    def bc_mid(ap2d):
        return ap2d.unsqueeze(1).broadcast_to([P, NT, E])

    st = {}
    st["partial"] = env.dram.tile([B + P, C], env.PDT, tag="partial",
                                  name="partial")

    nc.vector.tensor_tensor(
        out=glogA[:], in0=glogA[:], in1=bc_mid(env.bgbc[:]), op=TT.add)
    m1 = route_p.tile([P, NT], FP32, tag="m1", name="m1")
    nc.vector.reduce_max(m1[:], glogA[:], axis=mybir.AxisListType.X)
    expsA = gate_p.tile([P, NT, E], FP32, tag="expsA", name="expsA")
    nc.vector.tensor_tensor(
        out=expsA[:], in0=glogA[:], in1=bc_last(m1[:]), op=TT.subtract)
    nc.scalar.activation(
        expsA[:], expsA[:], mybir.ActivationFunctionType.Exp)
    seA = route_p.tile([P, NT], FP32, tag="seA", name="seA")
    nc.vector.reduce_sum(seA[:], expsA[:], axis=mybir.AxisListType.X)
    rsA = route_p.tile([P, NT], FP32, tag="rsA", name="rsA")
    nc.vector.reciprocal(rsA[:], seA[:])

    # top-2 membership via exp-space (same ordering as softmax probs)
    g1m = route_p.tile([P, NT], FP32, tag="g1m", name="g1m")
    nc.vector.reduce_max(g1m[:], expsA[:], axis=mybir.AxisListType.X)
    msk = gate_p.tile([P, NT, E], FP32, tag="scr", name="msk")
    nc.vector.tensor_tensor(
        out=msk[:], in0=expsA[:], in1=bc_last(g1m[:]), op=TT.is_ge)
    nc.vector.tensor_tensor(out=msk[:], in0=msk[:], in1=expsA[:], op=TT.mult)
    nc.vector.tensor_tensor(out=msk[:], in0=expsA[:], in1=msk[:], op=TT.subtract)
    g2m = route_p.tile([P, NT], FP32, tag="g2m", name="g2m")
    nc.vector.reduce_max(g2m[:], msk[:], axis=mybir.AxisListType.X)

    gbufs = [route_p.tile([P, NT], FP32, tag=f"gsel{j}", name=f"gsel{j}")
             for j in range(ELOC)]
    mU = [route_p.tile([P, NT], FP32, tag=f"mU{j}", name=f"mU{j}")
          for j in range(ELOC)]
    scrA = gate_p.tile([P, NT, E], FP32, tag="scr", name="scrA")
    for j in range(ELOC):
        nc.vector.tensor_tensor(
            out=scrA[:], in0=expsA[:], in1=bc_mid(env.selbc[:, j, :]),
            op=TT.mult)
        nc.vector.reduce_sum(gbufs[j][:], scrA[:], axis=mybir.AxisListType.X)
        nc.vector.tensor_tensor(
            out=mU[j][:], in0=gbufs[j][:], in1=g2m[:], op=TT.is_ge)
        nc.vector.tensor_mul(gbufs[j][:], gbufs[j][:], rsA[:])

    if env.upto == "A":
        return None

    # ------------------------------------------------ compaction
    rank = []
    tot = route_p.tile([P, ELOC], FP32, tag="tot", name="tot")
    for j in range(ELOC):
        inc = route_p.tile([P, NT], FP32, tag=f"inc{j}", name=f"inc{j}")
        nc.vector.tensor_tensor_scan(
            out=inc[:], data0=mU[j][:], data1=mU[j][:], initial=0.0,
            op0=TT.add, op1=TT.bypass,
        )
        exc = route_p.tile([P, NT], FP32, tag=f"exc{j}", name=f"exc{j}")
        nc.vector.tensor_sub(exc[:], inc[:], mU[j][:])
        nc.vector.tensor_copy(tot[:, j:j + 1], inc[:, NT - 1:NT])
        rank.append(exc)

    offp = env.pse.tile([P, ELOC], FP32, tag="offp", name="offp")
    nc.tensor.matmul(offp[:], lhsT=env.tri[:], rhs=tot[:], start=True, stop=True)
    offs = route_p.tile([P, ELOC], FP32, tag="offs", name="offs")
    nc.vector.tensor_copy(offs[:], offp[:])

    SENT = 48000.0
    idsb4 = env.persist.tile([P, ELOC * CAPT, 4], FP32, tag="idsb4",
                             name="idsb4")
    for j in range(ELOC):
        nc.vector.tensor_scalar(
            rank[j][:], rank[j][:], offs[:, j:j + 1], -SENT,
            op0=TT.add, op1=TT.add,
        )
        nc.vector.tensor_mul(rank[j][:], rank[j][:], mU[j][:])
        ri = route_p.tile([P, NT], INT32, tag="ri", name="ri")
        nc.vector.tensor_scalar(ri[:], rank[j][:], SENT, None, op0=TT.add)
        rhi_i = route_p.tile([P, NT], INT32, tag="rhi_i", name="rhi_i")
        nc.vector.tensor_scalar(
            rhi_i[:], ri[:], 7, None, op0=TT.logical_shift_right)
        rhi_f = route_p.tile([P, NT], FP32, tag="rhi_f", name="rhi_f")
        nc.vector.tensor_copy(rhi_f[:], rhi_i[:])
        rlo_i = route_p.tile([P, NT], INT32, tag="rlo_i", name="rlo_i")
        nc.vector.tensor_scalar(rlo_i[:], ri[:], 127, None, op0=TT.bitwise_and)
        rlo16 = route_p.tile([P, NT], FP16, tag="rlo16", name="rlo16")
        nc.vector.tensor_copy(rlo16[:], rlo_i[:])

        # ONE [P, NT, 128] one-hot per expert (lo part only; non-member rows
        # are neutralized by the hi compare baked into the metadata)
        ohlo = gate_p.tile([P, NT, P], BF16, tag="oh", name="ohlo", bufs=2)
        nc.vector.tensor_tensor(
            out=ohlo[:],
            in0=rlo16[:].unsqueeze(2).broadcast_to([P, NT, P]),
            in1=env.sio16[:].unsqueeze(1).broadcast_to([P, NT, P]),
            op=TT.is_equal,
        )

        m4 = route_p.tile([P, NT, 4], BF16, tag="meta4", name="m4")
        nc.vector.tensor_copy(m4[:, :, 0], env.fio_i[:])
        nc.vector.tensor_copy(m4[:, :, 1], env.pio_i[:])
        nc.vector.tensor_copy(m4[:, :, 2], gbufs[j][:])
        nc.vector.memset(m4[:, :, 3], 1.0)
        metaS = route_p.tile([P, NT, CAPT * 4], BF16, tag="metaS", name="metaS")
        eqhi = route_p.tile([P, NT], FP32, tag="eqhi", name="eqhi")
        for s in range(CAPT):
            nc.vector.tensor_scalar(
                eqhi[:], rhi_f[:], float(s), None, op0=TT.is_equal)
            nc.vector.tensor_tensor(
                out=metaS[:, :, s * 4:(s + 1) * 4],
                in0=m4[:],
                in1=eqhi[:].unsqueeze(2).broadcast_to([P, NT, 4]),
                op=TT.mult,
            )

        idp = env.pse.tile([P, CAPT * 4], FP32, tag="idp", name="idp")
        for ff in range(NT):
            nc.tensor.matmul(
                idp[:], lhsT=ohlo[:, ff, :], rhs=metaS[:, ff, :],
                start=(ff == 0), stop=(ff == NT - 1),
            )
        nc.scalar.copy(
            idsb4[:, j * CAPT:(j + 1) * CAPT, :],
            idp[:].rearrange("p (s c) -> p s c", c=4),
        )
    st["idsb4"] = idsb4

    NC2 = ELOC * CAPT
    tokf2 = route_p.tile([P, NC2], FP32, tag="tokf2", name="tokf2")
    nc.vector.tensor_scalar(
        tokf2[:], idsb4[:, :, 0], float(P), None, op0=TT.mult)
    nc.vector.tensor_add(tokf2[:], tokf2[:], idsb4[:, :, 1])
    idcast = env.persist.tile([P, NC2], INT32, tag="idcast", name="idcast")
    nc.vector.tensor_copy(idcast[:], tokf2[:])
    scatf = route_p.tile([P, NC2], FP32, tag="scatf", name="scatf")
    nc.vector.tensor_scalar(
        scatf[:], idsb4[:, :, 3], -float(TRASH), float(TRASH),
        op0=TT.mult, op1=TT.add,
    )
    nc.vector.tensor_add(scatf[:], scatf[:], tokf2[:])
    idscat = env.persist.tile([P, NC2], INT32, tag="idscat", name="idscat")
    nc.vector.tensor_copy(idscat[:], scatf[:])
    st["idscat"] = idscat

    if env.upto == "B":
        return None

    # ------------------------------------------------ gathers + transposes
    xdts = []
    for j in range(ELOC):
        xdt = env.xdtp.tile([P, DC, CAP], BF16, tag="xdt", name="xdt")
        for a in range(CAPT):
            xg = env.ffnp.tile([P, D], BF16, tag="xg", name="xg")
            nc.gpsimd.indirect_dma_start(
                out=xg[:],
                out_offset=None,
                in_=env.xb_in[:],
                in_offset=bass.IndirectOffsetOnAxis(
                    ap=idcast[:, j * CAPT + a:j * CAPT + a + 1], axis=0
                ),
            )
            # XBAR transpose: xts[p, c, s] = xg[s, c*128+p]; contiguous
            # staging tile (non-contiguous XBAR dst is broken on HW)
            # ACT copy: keeps the DVE FIFO clear so the PREVIOUS body's fc2
            # PSUM drains (DVE) are not queued behind gather latency
            xts = env.ffnp.tile([P, DC, P], BF16, tag="xts", name="xts")
            nc.sync.dma_start_transpose(xts[:], xg[:])
            nc.scalar.copy(xdt[:, :, a * P:(a + 1) * P], xts[:])
        xdts.append(xdt)
    st["xdts"] = xdts

    # ------------------------------------------------ weights + zero-init
    w1h, w2h = [], []
    for j in range(ELOC):
        pair = []
        for h in range(2):
            t = env.w1p.tile([P, DC // 2, H], BF16, tag="w1h", name="w1h")
            nc.sync.dma_start(
                t[:],
                env.w1_in[j, h * (D // 2):(h + 1) * (D // 2), :].rearrange(
                    "(c p) h -> p c h", p=P),
            )
            pair.append(t)
        w1h.append(pair)
    for j in range(ELOC):
        pair = []
        for h in range(2):
            t = env.w2p.tile([P, HC // 2, C], BF16, tag="w2h", name="w2h")
            nc.sync.dma_start(
                t[:],
                env.w2_in[j, h * (H // 2):(h + 1) * (H // 2), :].rearrange(
                    "(kc p) cc -> p kc cc", p=P),
            )
            pair.append(t)
        w2h.append(pair)
    st["w1h"], st["w2h"] = w1h, w2h

    partial = st["partial"]
    nzr = env.nzr
    for i in range(B // (nzr * P)):
        nc.scalar.dma_start(
            partial[i * nzr * P:(i + 1) * nzr * P, :].rearrange(
                "(a p) c -> p a c", p=P),
            env.zsb[:],
        )
    nc.scalar.dma_start(partial[B:B + P, :], env.zsb[:, 0, :])
    return st


def _emit_fc1(env, st, j):
    nc = env.nc
    if j == 0:
        st["hts"] = []
    hts = env.htp.tile([P, HC, CAP], BF16, tag="hts", name="hts")
    st["hts"].append(hts)
    xdt = st["xdts"][j]
    segs = [(0, 512), (512, CAP)]
    for m in range(HC):
        for s0, s1 in segs:
            hp = env.psum.tile([P, 512], FP32, tag="hp", name="hp")
            for ci in range(DC):
                src = st["w1h"][j][ci // (DC // 2)]
                nc.tensor.matmul(
                    hp[:, :s1 - s0],
                    lhsT=src[:, ci % (DC // 2), m * P:(m + 1) * P],
                    rhs=xdt[:, ci, s0:s1],
                    start=(ci == 0), stop=(ci == DC - 1),
                )
            nc.scalar.add(
                hts[:, m, s0:s1], hp[:, :s1 - s0], env.b1sb[:, j, m:m + 1]
            )


def _emit_fc2_combine(env, st):
    nc, TT = env.nc, env.TT
    partial = st["partial"]
    idsb4 = st["idsb4"]
    csegs = [(0, 512), (512, C)]
    for j in range(ELOC):
        hts = st["hts"][j]
        for a in range(CAPT):
            lps = []
            for s0, s1 in csegs:
                lp = env.psum.tile([P, 512], FP32, tag="lp", name="lp")
                for kc in range(HC):
                    src = st["w2h"][j][kc // (HC // 2)]
                    nc.tensor.matmul(
                        lp[:, :s1 - s0],
                        lhsT=hts[:, kc, a * P:(a + 1) * P],
                        rhs=src[:, kc % (HC // 2), s0:s1],
                        start=(kc == 0), stop=(kc == HC - 1),
                    )
                lps.append(lp)
            pst_sb = env.ffnp.tile([P, C], FP32, tag="logits", name="logits")
            for (s0, s1), lp in zip(csegs, lps):
                nc.vector.tensor_add(
                    pst_sb[:, s0:s1], lp[:, :s1 - s0], env.b2bc[:, j, s0:s1]
                )
            mx = env.ffnp.tile([P, 1], FP32, tag="mx", name="mx")
            nc.vector.reduce_max(mx[:], pst_sb[:], axis=mybir.AxisListType.X)
            nmx = env.ffnp.tile([P, 1], FP32, tag="nmx", name="nmx")
            nc.vector.tensor_scalar_mul(nmx[:], mx[:], -1.0)
            sevs = []
            for s0, s1 in csegs:
                ex = env.pse.tile([P, 512], FP32, tag="ex", name="ex")
                sv = env.ffnp.tile([P, 1], FP32, tag=f"sev{s0}", name="sv")
                nc.scalar.activation(
                    ex[:, :s1 - s0], pst_sb[:, s0:s1],
                    mybir.ActivationFunctionType.Exp,
                    bias=nmx[:], scale=1.0, accum_out=sv[:],
                )
                sevs.append(sv)
            sevsum = env.ffnp.tile([P, 1], FP32, tag="sevsum", name="sevsum")
            nc.vector.tensor_add(sevsum[:], sevs[0][:], sevs[1][:])
            lnz = env.ffnp.tile([P, 1], FP32, tag="lnz", name="lnz")
            nc.scalar.activation(
                lnz[:], sevsum[:], mybir.ActivationFunctionType.Ln,
            )
            total = env.ffnp.tile([P, 1], FP32, tag="total", name="total")
            nc.vector.tensor_add(total[:], mx[:], lnz[:])
            outsb = env.ffnp.tile([P, C], env.PDT, tag="outsb", name="outsb")
            nc.vector.tensor_scalar(
                outsb[:], pst_sb[:], total[:],
                idsb4[:, j * CAPT + a, 2:3],
                op0=TT.subtract, op1=TT.mult,
            )
            # expert 0 scatters onto freshly zeroed rows: plain writes (pad
            # rows write zeros to the trash row); expert 1 must accumulate
            nc.gpsimd.indirect_dma_start(
                out=partial[:],
                out_offset=bass.IndirectOffsetOnAxis(
                    ap=st["idscat"][:, j * CAPT + a:j * CAPT + a + 1], axis=0
                ),
                in_=outsb[:],
                in_offset=None,
                compute_op=(TT.bypass if j == 0 else TT.add),
            )

    if env.upto == "C":
        return None

    if env.combine == "rs":
        rsout = env.dram.tile([B // N_CORES, C], env.PDT, tag="rsout",
                              name="rsout")
        nc.gpsimd.collective_compute(
            "ReduceScatter",
            TT.add,
            replica_groups=[list(range(N_CORES))],
            ins=[partial[:B, :].opt()],
            outs=[rsout[:].opt()],
        )
        return rsout
    ot = env.ffnp.tile([P, C], FP32, tag="logits", name="otile")
    for i in range(B // P):
        nc.sync.dma_start(ot[:], partial[i * P:(i + 1) * P, :])
        nc.sync.dma_start(env.out_t[i * P:(i + 1) * P, :], ot[:])
    return None


# ---------------------------------------------------------------- host glue
_CACHE = {}


def _get_nc(combine: str):
    if combine not in _CACHE:
        _CACHE[combine] = build_bass(combine)
    return _CACHE[combine]


def make_in_maps(x, Wg, bg, W1, b1, W2, b2):
    import ml_dtypes
    bf16 = np.dtype(ml_dtypes.bfloat16)
    x = np.asarray(x, np.float32)
    xt = np.ascontiguousarray(x.T)
    xb = np.ascontiguousarray(x.astype(bf16))
    Wg = np.ascontiguousarray(np.asarray(Wg, np.float32))
    bg = np.tile(np.asarray(bg, np.float32).reshape(1, E), (P, 1))
    W1 = np.ascontiguousarray(np.asarray(W1, np.float32).astype(bf16))
    b1 = np.ascontiguousarray(np.asarray(b1, np.float32))
    W2 = np.ascontiguousarray(np.asarray(W2, np.float32).astype(bf16))
    b2 = np.ascontiguousarray(np.asarray(b2, np.float32))
    maps = []
    for i in range(N_CORES):
        lo = i * ELOC
        sel = np.zeros((ELOC, 1, E), np.float32)
        for j in range(ELOC):
            sel[j, 0, lo + j] = 1.0
        maps.append({
            "xt": xt,
            "xb": xb,
            "wg": Wg,
            "bg": bg,
            "w1": np.ascontiguousarray(W1[lo:lo + ELOC]),
            "b1": np.ascontiguousarray(b1[lo:lo + ELOC]),
            "w2": np.ascontiguousarray(W2[lo:lo + ELOC]),
            "b2": np.ascontiguousarray(
                np.tile(b2[lo:lo + ELOC].reshape(ELOC, 1, C), (1, P, 1))),
            "sel": np.ascontiguousarray(np.tile(sel, (1, P, 1))),
        })
    return maps


def _assert_capacity(x, Wg, bg):
    gate = np.asarray(x, np.float32) @ np.asarray(Wg, np.float32)
    gate += np.asarray(bg, np.float32).reshape(1, E)
    order = np.argsort(-gate, axis=1)[:, :TOPK]
    counts = np.bincount(order.ravel(), minlength=E)
    assert counts.max() <= CAP, (
        f"per-expert token load {counts.max()} exceeds CAP={CAP}; "
        f"raise CAP in kernel.py"
    )


def kernel(x, Wg, bg, W1, b1, W2, b2, k):
    assert int(k) == TOPK
    _assert_capacity(x, Wg, bg)
    combine = _COMBINE
    nc = _get_nc(combine)
    maps = make_in_maps(x, Wg, bg, W1, b1, W2, b2)
    res = run_bass_kernel_spmd(nc, maps, core_ids=list(range(N_CORES)))
    if combine == "rs":
        out = np.concatenate(
            [np.asarray(res.results[i]["out"], dtype=np.float32)
             for i in range(N_CORES)], axis=0)
    else:
        out = np.sum([res.results[i]["out"] for i in range(N_CORES)], axis=0)
    return out.astype(np.float32)


if __name__ == "__main__":
    rng = np.random.default_rng(0)
    x = rng.standard_normal((B, D), np.float32)
    Wg = rng.standard_normal((D, E), np.float32) / np.sqrt(D)
    bg = np.zeros((E,), np.float32)
    W1 = (rng.standard_normal((E, D, H)) / np.sqrt(D)).astype(np.float32)
    b1 = np.zeros((E, H), np.float32)
    W2 = (rng.standard_normal((E, H, C)) / np.sqrt(H)).astype(np.float32)
    b2 = np.zeros((E, C), np.float32)
    out = kernel(x, Wg, bg, W1, b1, W2, b2, 2)
    print("kernel ran, out:", out.shape, out.dtype, float(np.abs(out).max()))
